# Initial kernel scaffold
#
"""Trainium2 Bass kernel for a 2-layer RGCN (mean aggregation) + sigmoid scoring head.

Math (per reference):
  h1 = relu( sum_r mean_{e:(dst,r)} x[src] @ W1[r] + x @ root1 + b1 )
  h2 = relu( sum_r mean_{e:(dst,r)} h1[src] @ W2[r] + h1 @ root2 + b2 )
  out = sigmoid(h2 @ Ws + bs)

Strategy (8 NeuronCores, dst-sharded), v3:
  - All on-device data in bf16 (PSUM accumulation stays fp32): PE matmuls run
    at 1 cycle/row instead of 4 (fp32), DVE one-hot builds at 2x, and every
    DMA byte count halves.  Host-verified end-to-end rel err ~9e-4 (<< 2e-2).
  - Aggregate-then-transform: per (dst-block-of-128, relation), gather source
    rows per edge, segmented-sum via one-hot matmuls accumulating in PSUM
    (AT[f, d] += X[e, f]^T @ S[e, d], S one-hot with 1/cnt folded in), then
    transform: h[d, :] += AT_r^T @ W_r accumulated over relations.
  - Split-table gathers: int16 gather indices can't span 50000 rows, so edges
    are split into src<32768 (table x[0:32768]) and src>=32768 (table view
    x[N-32768:], idx -= N-32768).  Single-row 256B/512B gathers -- no pair
    overfetch.
  - Merged slot packing (v3.1): ALL (dst-block, relation) groups of a stream
    share ONE global slot run at max-over-cores exclusive-cumsum offsets --
    just one ceil-to-128 per stream, no per-group padding.  84->89% slot
    occupancy; gather descriptors are the measured bottleneck
    (~30ns/descriptor).  A group spanning a block boundary gets one one-hot
    build per touched block; foreign slots in the block are masked to -1 in
    that build's de column.
  - PSUM->SBUF copies + ReLU/Sigmoid run on the scalar(ACT) engine, keeping
    DVE free for one-hot builds; gathers round-robin the 4 SWDGE queues.
  - h1 is AllGather'd (bf16) across the 8 cores between layers.
"""

import os

import numpy as np
import ml_dtypes

import concourse.bacc as bacc
import concourse.bass as bass
import concourse.mybir as mybir
import concourse.tile as tile
import concourse.bass_utils as bass_utils

F32 = mybir.dt.float32
BF16 = mybir.dt.bfloat16
I16 = mybir.dt.int16
NPBF = ml_dtypes.bfloat16

NC = 8       # cores
LO = 32768   # lo gather table covers rows [0, LO); hi table rows [N-LO, N)


# ---------------------------------------------------------------------------
# Host-side scheduling
# ---------------------------------------------------------------------------

def _build_schedule(src, dst, etype, N, R, n_cores):
    """Partition edges by dst shard; per (dst-block, src-table stream) pack
    all R relations into one slot run at per-relation max-over-cores offsets.
    Emit the SPMD-common build list (one one-hot build per touched block per
    relation) and per-core slot arrays."""
    ND = N // n_cores                      # dst nodes per core
    NDB = (ND + 127) // 128                # dst blocks per core
    CH = NDB * R                           # (dst-block, relation) groups
    HOFF = N - LO

    seg = dst * R + etype
    cnt = np.bincount(seg, minlength=N * R)
    norm_all = (1.0 / np.maximum(cnt, 1)).astype(np.float32)[seg]

    core_of = dst // ND
    lens = np.zeros((2, n_cores, CH), np.int64)
    per_core = []
    for c in range(n_cores):
        m = core_of == c
        s = src[m]
        dl = dst[m] - c * ND
        t = etype[m]
        nv = norm_all[m]
        hi = s >= LO
        group = (dl >> 7) * R + t
        per_core.append((s, dl, nv, group, hi))
        lens[0, c] = np.bincount(group[~hi], minlength=CH)
        lens[1, c] = np.bincount(group[hi], minlength=CH)

    ml = lens.max(axis=1)                          # [2, CH] max len per group
    # one global slot run per stream: groups at exclusive-cumsum offsets,
    # only ONE ceil-to-128 per stream (no per-dst-block padding)
    off_flat = np.cumsum(ml, axis=1) - ml          # [2, CH] in-run offsets
    total = ml.sum(axis=1)                         # [2] run slot lengths
    NBLKLO = int(-(-total[0] // 128))
    NBLK = NBLKLO + int(-(-total[1] // 128))
    NSLOT = NBLK * 128
    mlv = ml.reshape(2, NDB, R)
    off_v = off_flat.reshape(2, NDB, R)

    # build (one-hot) enumeration in device order: db -> r -> st -> kb
    nkb = np.zeros((2, NDB, R), np.int64)
    colbase = np.zeros((2, CH), np.int64)
    ncol = 0
    for db in range(NDB):
        for r in range(R):
            for st in (0, 1):
                m = int(mlv[st, db, r])
                if m == 0:
                    continue
                o = int(off_v[st, db, r])
                k = (-(-(o + m) // 128)) - o // 128
                colbase[st, db * R + r] = ncol
                nkb[st, db, r] = k
                ncol += k

    colbase_flat = colbase
    # global slot base per stream: lo run first, then hi run
    slotbase_flat = np.zeros((2, CH), np.int64)
    slotbase_flat[1] = NBLKLO * 128

    cores = []
    for c in range(n_cores):
        s, dl, nv, group, hi = per_core[c]
        gidx = np.zeros(NSLOT, np.int16)
        de = np.full((128, ncol), -1.0, np.float32)
        nvs = np.zeros((128, ncol), np.float32)
        for st in (0, 1):
            msk = hi if st else ~hi
            ss, dls, nvv, g = s[msk], dl[msk], nv[msk], group[msk]
            order = np.argsort(g, kind="stable")
            ss, dls, nvv, g = ss[order], dls[order], nvv[order], g[order]
            cl = np.bincount(g, minlength=CH)
            gstart = np.zeros(CH, np.int64)
            gstart[1:] = np.cumsum(cl)[:-1]
            q = off_flat[st][g] + (np.arange(len(ss)) - gstart[g])  # in-run pos
            slot = slotbase_flat[st][g] + q
            gidx[slot] = (ss - (HOFF if st else 0)).astype(np.int16)
            col = colbase_flat[st][g] + (q // 128 - off_flat[st][g] // 128)
            de[q % 128, col] = (dls & 127).astype(np.float32)
            nvs[q % 128, col] = nvv

        idx16 = np.tile(gidx.reshape(NSLOT // 16, 16).T, (8, 1))  # [128, NSLOT/16]
        cores.append(dict(idx16=idx16, de=de, nv=nvs))

    return dict(ND=ND, NDB=NDB, CH=CH, ml=mlv, off=off_v, nkb=nkb,
                NBLKLO=NBLKLO, NBLK=NBLK, NSLOT=NSLOT, NCOL=ncol, cores=cores)


# ---------------------------------------------------------------------------
# Device program
# ---------------------------------------------------------------------------

_STAGE = int(os.environ.get("K_STAGE", "3"))  # 1=L1 only, 2=+allgather, 3=full
_NOGATHER = bool(int(os.environ.get("K_NOGATHER", "0")))
_NOONEHOT = bool(int(os.environ.get("K_NOONEHOT", "0")))


def _builds_for(sched, db, r):
    """SPMD-common list of (global block col) for (db, r), device order."""
    out = []
    for st in (0, 1):
        m = int(sched["ml"][st, db, r])
        if m == 0:
            continue
        o = int(sched["off"][st, db, r])
        kb0 = o // 128
        base = 0 if st == 0 else sched["NBLKLO"]
        for kb in range(kb0, kb0 + int(sched["nkb"][st, db, r])):
            out.append(base + kb)
    return out


def _build_program(N, F, H, O, R, n_cores, sched, G1=64, G2=64):
    ND, NDB = sched["ND"], sched["NDB"]
    NBLKLO, NBLK, NSLOT = sched["NBLKLO"], sched["NBLK"], sched["NSLOT"]
    NCOL = sched["NCOL"]
    NBLKHI = NBLK - NBLKLO
    NDP = NDB * 128  # padded dst count per core
    HOFF = N - LO

    nc = bacc.Bacc("TRN2", target_bir_lowering=False, debug=False,
                   num_devices=n_cores, num_swdge_queues=4)

    # ---- I/O ----
    x_d = nc.dram_tensor("x", [N, F], BF16, kind="ExternalInput")
    xt_d = nc.dram_tensor("xt", [128, NDP], BF16, kind="ExternalInput")
    idx_d = nc.dram_tensor("idx16", [128, NSLOT // 16], I16, kind="ExternalInput")
    de_d = nc.dram_tensor("de", [128, NCOL], F32, kind="ExternalInput")
    nv_d = nc.dram_tensor("nv", [128, NCOL], F32, kind="ExternalInput")
    w1_d = nc.dram_tensor("w1", [128, R * H], BF16, kind="ExternalInput")
    w2_d = nc.dram_tensor("w2", [128, R * 2 * O], BF16, kind="ExternalInput")
    r1_d = nc.dram_tensor("r1", [128, H], BF16, kind="ExternalInput")
    r2_d = nc.dram_tensor("r2", [128, 2 * O], BF16, kind="ExternalInput")
    b1_d = nc.dram_tensor("b1", [1, H], BF16, kind="ExternalInput")
    b2_d = nc.dram_tensor("b2", [1, O], BF16, kind="ExternalInput")
    ws_d = nc.dram_tensor("ws", [128, 1], BF16, kind="ExternalInput")
    bs_d = nc.dram_tensor("bs", [1, 1], F32, kind="ExternalInput")
    io_d = nc.dram_tensor("iota", [128, 128], BF16, kind="ExternalInput")
    id_d = nc.dram_tensor("ident", [128, 128], BF16, kind="ExternalInput")
    sc_d = nc.dram_tensor("scores", [1, NDP], F32, kind="ExternalOutput")

    eq, mul = mybir.AluOpType.is_equal, mybir.AluOpType.mult
    ACopy = mybir.ActivationFunctionType.Copy
    ARelu = mybir.ActivationFunctionType.Relu
    ASig = mybir.ActivationFunctionType.Sigmoid

    with tile.TileContext(nc) as tc:
        with (
            tc.tile_pool(name="const", bufs=1) as cp,
            tc.tile_pool(name="dram", bufs=1, space="DRAM") as dramp,
        ):
            def load_const(d, shape, dtype=BF16):
                t = cp.tile(shape, dtype, tag=d.name)
                nc.sync.dma_start(t[:], d[:])
                return t

            idx_s = load_const(idx_d, [128, NSLOT // 16], I16)
            de_s = load_const(de_d, [128, NCOL], F32)
            nv_s = load_const(nv_d, [128, NCOL], F32)
            w1_s = load_const(w1_d, [128, R * H])
            w2_s = load_const(w2_d, [128, R * 2 * O])
            r1_s = load_const(r1_d, [128, H])
            r2_s = load_const(r2_d, [128, 2 * O])
            b1_s = load_const(b1_d, [1, H])
            b2_s = load_const(b2_d, [1, O])
            ws_s = load_const(ws_d, [128, 1])
            bs_s = load_const(bs_d, [1, 1], F32)
            io_s = load_const(io_d, [128, 128])
            id_s = load_const(id_d, [128, 128])
            xt_s = load_const(xt_d, [128, NDP])
            ones1 = cp.tile([1, 128], BF16, tag="ones1")
            nc.vector.memset(ones1[:], 1.0)
            dummy = cp.tile([128, 2 * H], BF16, tag="dummy")
            if _NOGATHER:
                nc.vector.memset(dummy[:], 0.0)

            h1loc = dramp.tile([NDP, H], BF16)      # this core's h1 rows (padded)
            if bool(int(os.environ.get("K_SHARED", "1"))):
                h1full = dramp.tile([N, H], BF16, addr_space="Shared")
            else:
                h1full = dramp.tile([N, H], BF16)   # allgathered h1

            qn = [0]

            def make_gather(ringp, lo_ap, hi_ap, elem, G, bufs):
                ring = {}

                def get(b):  # b = global block column
                    if _NOGATHER:
                        return dummy, 0
                    st = 0 if b < NBLKLO else 1
                    s0 = 0 if st == 0 else NBLKLO
                    NS = NBLKLO if st == 0 else NBLKHI
                    rel = b - s0
                    cb = rel // G
                    off = (rel % G) * elem
                    key = (st, cb)
                    if key in ring:
                        return ring[key], off
                    w = min(G, NS - cb * G)
                    t = ringp.tile([128, G * elem], BF16, tag=f"xr{st}",
                                   bufs=bufs)
                    col0 = (s0 + cb * G) * 8
                    nc.gpsimd.dma_gather(
                        t[:, : w * elem].rearrange("p (g f) -> p g f", f=elem),
                        lo_ap if st == 0 else hi_ap,
                        idx_s[:, col0: col0 + w * 8],
                        w * 128,
                        w * 128,
                        elem,
                        single_packet=False,
                        queue_num=qn[0] % 4,
                    )
                    qn[0] += 1
                    ring[key] = t
                    return t, off
                return get

            # AllGather plumbing (chunk hooks kept; default single collective)
            h1f3 = h1full[:].rearrange("(c n) h -> c n h", c=n_cores)
            ag_done = [0]

            def allgather_to(db_end):
                r0, r1 = ag_done[0] * 128, min(db_end * 128, ND)
                if r1 <= r0:
                    return
                out_ap = h1full[:] if (r0 == 0 and r1 == ND) else h1f3[:, r0:r1, :]
                nc.gpsimd.collective_compute(
                    "AllGather",
                    mybir.AluOpType.bypass,
                    replica_groups=[list(range(n_cores))],
                    ins=[h1loc[r0:r1, :].opt()],
                    outs=[out_ap.opt()],
                )
                ag_done[0] = db_end

            _AGC = int(os.environ.get("K_AGCHUNK", "1"))
            ag_marks = {(NDB * (i + 1)) // _AGC for i in range(_AGC)} if _AGC > 1 \
                else {NDB}

            # =============== LAYER 1 ===============
            with (
                tc.tile_pool(name="ring1", bufs=3) as ringp,
                tc.tile_pool(name="s1", bufs=8) as sp,
                tc.tile_pool(name="at1", bufs=4) as atp,
                tc.tile_pool(name="h1sb", bufs=2) as h1p,
                tc.tile_pool(name="pat1", bufs=3, space="PSUM") as patp,
                tc.tile_pool(name="ph1", bufs=2, space="PSUM") as php,
            ):
                get1 = make_gather(ringp, x_d[0:LO, :], x_d[HOFF:N, :], F, G1, 3)

                col = 0
                for db in range(NDB):
                    psum_h = php.tile([128, H], F32)
                    nc.tensor.matmul(psum_h[:], xt_s[:, db * 128:(db + 1) * 128],
                                     r1_s[:], start=True, stop=False)
                    for r in range(R):
                        builds = _builds_for(sched, db, r)
                        tot = len(builds)
                        if tot == 0:
                            continue
                        psum_at = patp.tile([128, 128], F32)
                        for k, b in enumerate(builds):
                            xr, off = get1(b)
                            if _NOONEHOT:
                                se = io_s
                            else:
                                se = sp.tile([128, 128], BF16, tag="se")
                                nc.vector.tensor_scalar(
                                    se[:], io_s[:], de_s[:, col:col + 1],
                                    nv_s[:, col:col + 1], op0=eq, op1=mul)
                            col += 1
                            nc.tensor.matmul(psum_at[:], xr[:, off:off + F],
                                             se[:], start=(k == 0),
                                             stop=(k == tot - 1))
                        at_sb = atp.tile([128, 128], BF16)
                        nc.scalar.activation(at_sb[:], psum_at[:], ACopy)
                        nc.tensor.matmul(psum_h[:], at_sb[:],
                                         w1_s[:, r * H:(r + 1) * H],
                                         start=False, stop=False)
                    nc.tensor.matmul(psum_h[:], ones1[:], b1_s[:],
                                     start=False, stop=True)
                    h1_sb = h1p.tile([128, H], BF16)
                    nc.scalar.activation(h1_sb[:], psum_h[:], ARelu)
                    nc.sync.dma_start(h1loc[db * 128:(db + 1) * 128, :], h1_sb[:])
                    if _STAGE >= 2 and (db + 1) in ag_marks:
                        allgather_to(db + 1)
                    if _STAGE < 3:
                        sc_sb0 = h1p.tile([1, 128], F32, tag="scdbg")
                        nc.vector.tensor_copy(sc_sb0[:], h1_sb[0:1, 0:128])
                        nc.sync.dma_start(sc_d[0:1, db * 128:(db + 1) * 128],
                                          sc_sb0[:])

            # =============== ALLGATHER h1 (any remainder) ===============
            if _STAGE >= 2:
                allgather_to(NDB)

            # =============== LAYER 2 ===============
            if _STAGE >= 3:
              with (
                  tc.tile_pool(name="ring2", bufs=2) as ringp2,
                  tc.tile_pool(name="s2", bufs=8) as sp2,
                  tc.tile_pool(name="at2", bufs=4) as atp2,
                  tc.tile_pool(name="h2sb", bufs=2) as h2p,
                  tc.tile_pool(name="misc2", bufs=2) as mp2,
                  tc.tile_pool(name="pat2lo", bufs=2, space="PSUM") as patlo,
                  tc.tile_pool(name="pat2hi", bufs=2, space="PSUM") as pathi,
                  tc.tile_pool(name="ph2", bufs=2, space="PSUM") as php2,
                  tc.tile_pool(name="pmisc", bufs=1, space="PSUM") as pmp,
              ):
                  get2 = make_gather(ringp2, h1full[0:LO, :], h1full[HOFF:N, :],
                                     H, G2, 2)

                  col = 0
                  for db in range(NDB):
                      # root2 term needs h1^T of this dst block
                      h1row = mp2.tile([128, H], BF16, tag="h1row")
                      nc.sync.dma_start(h1row[:], h1loc[db * 128:(db + 1) * 128, :])
                      psum_h2 = php2.tile([128, O], F32)
                      h1t = []
                      for h in range(2):
                          pt = pmp.tile([128, 128], BF16, tag="ptr")
                          nc.tensor.transpose(pt[:], h1row[:, h * 128:(h + 1) * 128],
                                              id_s[:])
                          ht = mp2.tile([128, 128], BF16, tag=f"h1t{h}")
                          nc.scalar.activation(ht[:], pt[:], ACopy)
                          h1t.append(ht)
                      nc.tensor.matmul(psum_h2[:], h1t[0][:], r2_s[:, 0:O],
                                       start=True, stop=False)
                      nc.tensor.matmul(psum_h2[:], h1t[1][:], r2_s[:, O:2 * O],
                                       start=False, stop=False)

                      for r in range(R):
                          builds = _builds_for(sched, db, r)
                          tot = len(builds)
                          if tot == 0:
                              continue
                          at_lo = patlo.tile([128, 128], F32)
                          at_hi = pathi.tile([128, 128], F32)
                          for k, b in enumerate(builds):
                              xr, off = get2(b)
                              if _NOONEHOT:
                                  se = io_s
                              else:
                                  se = sp2.tile([128, 128], BF16, tag="se2")
                                  nc.vector.tensor_scalar(
                                      se[:], io_s[:], de_s[:, col:col + 1],
                                      nv_s[:, col:col + 1], op0=eq, op1=mul)
                              col += 1
                              st_f, sp_f = (k == 0), (k == tot - 1)
                              nc.tensor.matmul(at_lo[:], xr[:, off:off + 128],
                                               se[:], start=st_f, stop=sp_f)
                              nc.tensor.matmul(at_hi[:], xr[:, off + 128:off + 256],
                                               se[:], start=st_f, stop=sp_f)
                          at_sb = atp2.tile([128, 2 * 128], BF16)
                          nc.scalar.activation(at_sb[:, 0:128], at_lo[:], ACopy)
                          nc.scalar.activation(at_sb[:, 128:256], at_hi[:], ACopy)
                          for h in range(2):
                              nc.tensor.matmul(
                                  psum_h2[:], at_sb[:, h * 128:(h + 1) * 128],
                                  w2_s[:, (r * 2 + h) * O:(r * 2 + h + 1) * O],
                                  start=False, stop=False)
                      nc.tensor.matmul(psum_h2[:], ones1[:], b2_s[:],
                                       start=False, stop=True)
                      h2_sb = h2p.tile([128, O], BF16)
                      nc.scalar.activation(h2_sb[:], psum_h2[:], ARelu)

                      # head: scores = sigmoid(h2 @ Ws + bs)
                      pt2 = pmp.tile([128, 128], BF16, tag="ptr")
                      nc.tensor.transpose(pt2[:], h2_sb[:], id_s[:])
                      h2t = mp2.tile([128, 128], BF16, tag="h2t")
                      nc.scalar.activation(h2t[:], pt2[:], ACopy)
                      psc = pmp.tile([1, 128], F32, tag="psc")
                      nc.tensor.matmul(psc[:], ws_s[:], h2t[:], start=True, stop=True)
                      sc_sb = mp2.tile([1, 128], F32, tag="scsb")
                      nc.scalar.activation(sc_sb[:], psc[:], ASig,
                                           bias=bs_s[0:1, 0:1])
                      nc.sync.dma_start(sc_d[0:1, db * 128:(db + 1) * 128], sc_sb[:])

    nc.compile()
    return nc


# ---------------------------------------------------------------------------
# Entry point
# ---------------------------------------------------------------------------

def kernel(x, edge_index, edge_type, W1, root1, b1, W2, root2, b2, Ws, bs):
    x = np.ascontiguousarray(np.asarray(x, np.float32))
    ei = np.asarray(edge_index)
    et = np.asarray(edge_type).astype(np.int64)
    src, dst = ei[0].astype(np.int64), ei[1].astype(np.int64)
    W1 = np.asarray(W1, np.float32)
    root1 = np.ascontiguousarray(np.asarray(root1, np.float32))
    b1 = np.asarray(b1, np.float32)
    W2 = np.asarray(W2, np.float32)
    root2 = np.asarray(root2, np.float32)
    b2 = np.asarray(b2, np.float32)
    Ws = np.ascontiguousarray(np.asarray(Ws, np.float32))
    bs = np.asarray(bs, np.float32)

    N, F = x.shape
    R, _, H = W1.shape
    O = W2.shape[2]

    sched = _build_schedule(src, dst, et, N, R, NC)
    ND, NDB = sched["ND"], sched["NDB"]
    NDP = NDB * 128

    nc = _build_program(N, F, H, O, R, NC, sched)

    # common (replicated) inputs
    xbf = x.astype(NPBF)
    w1f = np.concatenate([W1[r] for r in range(R)], axis=1)            # [F, R*H]
    w2f = np.concatenate(
        [W2[r][h * 128:(h + 1) * 128, :] for r in range(R) for h in range(2)],
        axis=1)                                                         # [128, R*2*O]
    r2f = np.concatenate([root2[0:128, :], root2[128:256, :]], axis=1)  # [128, 2*O]
    iota = np.tile(np.arange(128, dtype=np.float32), (128, 1))
    ident = np.eye(128, dtype=np.float32)

    common = dict(
        x=np.ascontiguousarray(xbf),
        w1=np.ascontiguousarray(w1f.astype(NPBF)),
        w2=np.ascontiguousarray(w2f.astype(NPBF)),
        r1=root1.astype(NPBF), r2=np.ascontiguousarray(r2f.astype(NPBF)),
        b1=np.ascontiguousarray(b1.reshape(1, H).astype(NPBF)),
        b2=np.ascontiguousarray(b2.reshape(1, O).astype(NPBF)),
        ws=Ws.astype(NPBF), bs=np.ascontiguousarray(bs.reshape(1, 1)),
        iota=np.ascontiguousarray(iota.astype(NPBF)),
        ident=ident.astype(NPBF),
    )

    in_maps = []
    for c in range(NC):
        xt = np.zeros((128, NDP), NPBF)
        xt[:, :ND] = xbf[c * ND:(c + 1) * ND].T
        m = dict(common)
        m.update(
            xt=xt,
            idx16=np.ascontiguousarray(sched["cores"][c]["idx16"]),
            de=np.ascontiguousarray(sched["cores"][c]["de"]),
            nv=np.ascontiguousarray(sched["cores"][c]["nv"]),
        )
        in_maps.append(m)

    trace = bool(int(os.environ.get("K_TRACE", "0")))
    res = bass_utils.run_bass_kernel_spmd(nc, in_maps, core_ids=list(range(NC)),
                                          trace=trace)
    global last_exec_time_ns, last_results, last_nc, last_in_maps
    last_results = res
    last_exec_time_ns = res.exec_time_ns
    last_nc = nc
    last_in_maps = in_maps
    out = np.concatenate(
        [res.results[c]["scores"][0, :ND] for c in range(NC)])
    return out.astype(np.float32)


if __name__ == "__main__":
    import reference
    inputs = {k: np.asarray(v) for k, v in reference.setup_inputs().items()}
    got = kernel(**inputs)
    exp = np.asarray(reference.reference(**{k: v for k, v in reference.setup_inputs().items()}))
    err = np.abs(got - exp).max()
    rel = np.linalg.norm(got - exp) / np.linalg.norm(exp)
    print(f"max abs err {err:.3e}  rel {rel:.3e}")



# revision 1
# speedup vs baseline: 1.0096x; 1.0096x over previous
"""Trainium2 Bass kernel for a 2-layer RGCN (mean aggregation) + sigmoid scoring head.

Math (per reference):
  h1 = relu( sum_r mean_{e:(dst,r)} x[src] @ W1[r] + x @ root1 + b1 )
  h2 = relu( sum_r mean_{e:(dst,r)} h1[src] @ W2[r] + h1 @ root2 + b2 )
  out = sigmoid(h2 @ Ws + bs)

Strategy (8 NeuronCores, dst-sharded), v3:
  - All on-device data in bf16 (PSUM accumulation stays fp32): PE matmuls run
    at 1 cycle/row instead of 4 (fp32), DVE one-hot builds at 2x, and every
    DMA byte count halves.  Host-verified end-to-end rel err ~9e-4 (<< 2e-2).
  - Aggregate-then-transform: per (dst-block-of-128, relation), gather source
    rows per edge, segmented-sum via one-hot matmuls accumulating in PSUM
    (AT[f, d] += X[e, f]^T @ S[e, d], S one-hot with 1/cnt folded in), then
    transform: h[d, :] += AT_r^T @ W_r accumulated over relations.
  - Split-table gathers: int16 gather indices can't span 50000 rows, so edges
    are split into src<32768 (table x[0:32768]) and src>=32768 (table view
    x[N-32768:], idx -= N-32768).  Single-row 256B/512B gathers -- no pair
    overfetch.
  - Merged slot packing (v3.1): ALL (dst-block, relation) groups of a stream
    share ONE global slot run at max-over-cores exclusive-cumsum offsets --
    just one ceil-to-128 per stream, no per-group padding.  84->89% slot
    occupancy; gather descriptors are the measured bottleneck
    (~30ns/descriptor).  A group spanning a block boundary gets one one-hot
    build per touched block; foreign slots in the block are masked to -1 in
    that build's de column.
  - PSUM->SBUF copies + ReLU/Sigmoid run on the scalar(ACT) engine, keeping
    DVE free for one-hot builds; gathers round-robin the 4 SWDGE queues.
  - h1 is AllGather'd (bf16) across the 8 cores between layers.
"""

import os

import numpy as np
import ml_dtypes

import concourse.bacc as bacc
import concourse.bass as bass
import concourse.mybir as mybir
import concourse.tile as tile
import concourse.bass_utils as bass_utils

F32 = mybir.dt.float32
BF16 = mybir.dt.bfloat16
I16 = mybir.dt.int16
NPBF = ml_dtypes.bfloat16

NC = 8       # cores
LO = 32768   # lo gather table covers rows [0, LO); hi table rows [N-LO, N)


# ---------------------------------------------------------------------------
# Host-side scheduling
# ---------------------------------------------------------------------------

def _build_schedule(src, dst, etype, N, R, n_cores):
    """Partition edges by dst shard; per (dst-block, src-table stream) pack
    all R relations into one slot run at per-relation max-over-cores offsets.
    Emit the SPMD-common build list (one one-hot build per touched block per
    relation) and per-core slot arrays."""
    ND = N // n_cores                      # dst nodes per core
    NDB = (ND + 127) // 128                # dst blocks per core
    CH = NDB * R                           # (dst-block, relation) groups
    HOFF = N - LO

    seg = dst * R + etype
    cnt = np.bincount(seg, minlength=N * R)
    norm_all = (1.0 / np.maximum(cnt, 1)).astype(np.float32)[seg]

    core_of = dst // ND
    lens = np.zeros((2, n_cores, CH), np.int64)
    per_core = []
    for c in range(n_cores):
        m = core_of == c
        s = src[m]
        dl = dst[m] - c * ND
        t = etype[m]
        nv = norm_all[m]
        hi = s >= LO
        group = (dl >> 7) * R + t
        per_core.append((s, dl, nv, group, hi))
        lens[0, c] = np.bincount(group[~hi], minlength=CH)
        lens[1, c] = np.bincount(group[hi], minlength=CH)

    ml = lens.max(axis=1)                          # [2, CH] max len per group
    # one global slot run per stream: groups at exclusive-cumsum offsets,
    # only ONE ceil-to-128 per stream (no per-dst-block padding)
    off_flat = np.cumsum(ml, axis=1) - ml          # [2, CH] in-run offsets
    total = ml.sum(axis=1)                         # [2] run slot lengths
    NBLKLO = int(-(-total[0] // 128))
    NBLK = NBLKLO + int(-(-total[1] // 128))
    NSLOT = NBLK * 128
    mlv = ml.reshape(2, NDB, R)
    off_v = off_flat.reshape(2, NDB, R)

    # build (one-hot) enumeration in device order: db -> r -> st -> kb
    nkb = np.zeros((2, NDB, R), np.int64)
    colbase = np.zeros((2, CH), np.int64)
    ncol = 0
    for db in range(NDB):
        for r in range(R):
            for st in (0, 1):
                m = int(mlv[st, db, r])
                if m == 0:
                    continue
                o = int(off_v[st, db, r])
                k = (-(-(o + m) // 128)) - o // 128
                colbase[st, db * R + r] = ncol
                nkb[st, db, r] = k
                ncol += k

    colbase_flat = colbase
    # global slot base per stream: lo run first, then hi run
    slotbase_flat = np.zeros((2, CH), np.int64)
    slotbase_flat[1] = NBLKLO * 128

    cores = []
    for c in range(n_cores):
        s, dl, nv, group, hi = per_core[c]
        gidx = np.zeros(NSLOT, np.int16)
        de = np.full((128, ncol), -1.0, np.float32)
        nvs = np.zeros((128, ncol), np.float32)
        for st in (0, 1):
            msk = hi if st else ~hi
            ss, dls, nvv, g = s[msk], dl[msk], nv[msk], group[msk]
            order = np.argsort(g, kind="stable")
            ss, dls, nvv, g = ss[order], dls[order], nvv[order], g[order]
            cl = np.bincount(g, minlength=CH)
            gstart = np.zeros(CH, np.int64)
            gstart[1:] = np.cumsum(cl)[:-1]
            q = off_flat[st][g] + (np.arange(len(ss)) - gstart[g])  # in-run pos
            slot = slotbase_flat[st][g] + q
            gidx[slot] = (ss - (HOFF if st else 0)).astype(np.int16)
            col = colbase_flat[st][g] + (q // 128 - off_flat[st][g] // 128)
            de[q % 128, col] = (dls & 127).astype(np.float32)
            nvs[q % 128, col] = nvv

        idx16 = np.tile(gidx.reshape(NSLOT // 16, 16).T, (8, 1))  # [128, NSLOT/16]
        cores.append(dict(idx16=idx16, de=de, nv=nvs))

    return dict(ND=ND, NDB=NDB, CH=CH, ml=mlv, off=off_v, nkb=nkb,
                NBLKLO=NBLKLO, NBLK=NBLK, NSLOT=NSLOT, NCOL=ncol, cores=cores)


# ---------------------------------------------------------------------------
# Device program
# ---------------------------------------------------------------------------

_STAGE = int(os.environ.get("K_STAGE", "3"))  # 1=L1 only, 2=+allgather, 3=full
_NOGATHER = bool(int(os.environ.get("K_NOGATHER", "0")))
_NOONEHOT = bool(int(os.environ.get("K_NOONEHOT", "0")))


def _builds_for(sched, db, r):
    """SPMD-common list of (global block col) for (db, r), device order."""
    out = []
    for st in (0, 1):
        m = int(sched["ml"][st, db, r])
        if m == 0:
            continue
        o = int(sched["off"][st, db, r])
        kb0 = o // 128
        base = 0 if st == 0 else sched["NBLKLO"]
        for kb in range(kb0, kb0 + int(sched["nkb"][st, db, r])):
            out.append(base + kb)
    return out


def _build_program(N, F, H, O, R, n_cores, sched, G1=64, G2=64):
    ND, NDB = sched["ND"], sched["NDB"]
    NBLKLO, NBLK, NSLOT = sched["NBLKLO"], sched["NBLK"], sched["NSLOT"]
    NCOL = sched["NCOL"]
    NBLKHI = NBLK - NBLKLO
    NDP = NDB * 128  # padded dst count per core
    HOFF = N - LO

    nc = bacc.Bacc("TRN2", target_bir_lowering=False, debug=False,
                   num_devices=n_cores, num_swdge_queues=4)

    # ---- I/O ----
    x_d = nc.dram_tensor("x", [N, F], BF16, kind="ExternalInput")
    xt_d = nc.dram_tensor("xt", [128, NDP], BF16, kind="ExternalInput")
    idx_d = nc.dram_tensor("idx16", [128, NSLOT // 16], I16, kind="ExternalInput")
    de_d = nc.dram_tensor("de", [128, NCOL], F32, kind="ExternalInput")
    nv_d = nc.dram_tensor("nv", [128, NCOL], F32, kind="ExternalInput")
    w1_d = nc.dram_tensor("w1", [128, R * H], BF16, kind="ExternalInput")
    w2_d = nc.dram_tensor("w2", [128, R * 2 * O], BF16, kind="ExternalInput")
    r1_d = nc.dram_tensor("r1", [128, H], BF16, kind="ExternalInput")
    r2_d = nc.dram_tensor("r2", [128, 2 * O], BF16, kind="ExternalInput")
    b1_d = nc.dram_tensor("b1", [1, H], BF16, kind="ExternalInput")
    b2_d = nc.dram_tensor("b2", [1, O], BF16, kind="ExternalInput")
    ws_d = nc.dram_tensor("ws", [128, 1], BF16, kind="ExternalInput")
    bs_d = nc.dram_tensor("bs", [1, 1], F32, kind="ExternalInput")
    io_d = nc.dram_tensor("iota", [128, 128], BF16, kind="ExternalInput")
    id_d = nc.dram_tensor("ident", [128, 128], BF16, kind="ExternalInput")
    sc_d = nc.dram_tensor("scores", [1, NDP], F32, kind="ExternalOutput")

    eq, mul = mybir.AluOpType.is_equal, mybir.AluOpType.mult
    ACopy = mybir.ActivationFunctionType.Copy
    ARelu = mybir.ActivationFunctionType.Relu
    ASig = mybir.ActivationFunctionType.Sigmoid

    with tile.TileContext(nc) as tc:
        with (
            tc.tile_pool(name="const", bufs=1) as cp,
            tc.tile_pool(name="dram", bufs=1, space="DRAM") as dramp,
        ):
            def load_const(d, shape, dtype=BF16):
                t = cp.tile(shape, dtype, tag=d.name)
                nc.sync.dma_start(t[:], d[:])
                return t

            idx_s = load_const(idx_d, [128, NSLOT // 16], I16)
            de_s = load_const(de_d, [128, NCOL], F32)
            nv_s = load_const(nv_d, [128, NCOL], F32)
            w1_s = load_const(w1_d, [128, R * H])
            w2_s = load_const(w2_d, [128, R * 2 * O])
            r1_s = load_const(r1_d, [128, H])
            r2_s = load_const(r2_d, [128, 2 * O])
            b1_s = load_const(b1_d, [1, H])
            b2_s = load_const(b2_d, [1, O])
            ws_s = load_const(ws_d, [128, 1])
            bs_s = load_const(bs_d, [1, 1], F32)
            io_s = load_const(io_d, [128, 128])
            id_s = load_const(id_d, [128, 128])
            xt_s = load_const(xt_d, [128, NDP])
            ones1 = cp.tile([1, 128], BF16, tag="ones1")
            nc.vector.memset(ones1[:], 1.0)
            dummy = cp.tile([128, 2 * H], BF16, tag="dummy")
            if _NOGATHER:
                nc.vector.memset(dummy[:], 0.0)

            h1loc = dramp.tile([NDP, H], BF16)      # this core's h1 rows (padded)
            if bool(int(os.environ.get("K_SHARED", "1"))):
                h1full = dramp.tile([N, H], BF16, addr_space="Shared")
            else:
                h1full = dramp.tile([N, H], BF16)   # allgathered h1

            qn = [0]

            def make_gather(ringp, lo_ap, hi_ap, elem, G, bufs):
                ring = {}

                def get(b):  # b = global block column
                    if _NOGATHER:
                        return dummy, 0
                    st = 0 if b < NBLKLO else 1
                    s0 = 0 if st == 0 else NBLKLO
                    NS = NBLKLO if st == 0 else NBLKHI
                    rel = b - s0
                    cb = rel // G
                    off = (rel % G) * elem
                    key = (st, cb)
                    if key in ring:
                        return ring[key], off
                    w = min(G, NS - cb * G)
                    t = ringp.tile([128, G * elem], BF16, tag=f"xr{st}",
                                   bufs=bufs)
                    col0 = (s0 + cb * G) * 8
                    nc.gpsimd.dma_gather(
                        t[:, : w * elem].rearrange("p (g f) -> p g f", f=elem),
                        lo_ap if st == 0 else hi_ap,
                        idx_s[:, col0: col0 + w * 8],
                        w * 128,
                        w * 128,
                        elem,
                        single_packet=False,
                        queue_num=qn[0] % 4,
                    )
                    qn[0] += 1
                    ring[key] = t
                    return t, off
                return get

            # AllGather plumbing (chunk hooks kept; default single collective)
            h1f3 = h1full[:].rearrange("(c n) h -> c n h", c=n_cores)
            ag_done = [0]

            def allgather_to(db_end):
                r0, r1 = ag_done[0] * 128, min(db_end * 128, ND)
                if r1 <= r0:
                    return
                out_ap = h1full[:] if (r0 == 0 and r1 == ND) else h1f3[:, r0:r1, :]
                nc.gpsimd.collective_compute(
                    "AllGather",
                    mybir.AluOpType.bypass,
                    replica_groups=[list(range(n_cores))],
                    ins=[h1loc[r0:r1, :].opt()],
                    outs=[out_ap.opt()],
                )
                ag_done[0] = db_end

            _AGC = int(os.environ.get("K_AGCHUNK", "1"))
            ag_marks = {(NDB * (i + 1)) // _AGC for i in range(_AGC)} if _AGC > 1 \
                else {NDB}

            # =============== LAYER 1 ===============
            with (
                tc.tile_pool(name="ring1", bufs=3) as ringp,
                tc.tile_pool(name="s1", bufs=8) as sp,
                tc.tile_pool(name="at1", bufs=4) as atp,
                tc.tile_pool(name="h1sb", bufs=2) as h1p,
                tc.tile_pool(name="pat1", bufs=3, space="PSUM") as patp,
                tc.tile_pool(name="ph1", bufs=2, space="PSUM") as php,
            ):
                get1 = make_gather(ringp, x_d[0:LO, :], x_d[HOFF:N, :], F, G1, 3)

                col = 0
                for db in range(NDB):
                    psum_h = php.tile([128, H], F32)
                    nc.tensor.matmul(psum_h[:], xt_s[:, db * 128:(db + 1) * 128],
                                     r1_s[:], start=True, stop=False)
                    for r in range(R):
                        builds = _builds_for(sched, db, r)
                        tot = len(builds)
                        if tot == 0:
                            continue
                        psum_at = patp.tile([128, 128], F32)
                        for k, b in enumerate(builds):
                            xr, off = get1(b)
                            if _NOONEHOT:
                                se = io_s
                            else:
                                se = sp.tile([128, 128], BF16, tag="se")
                                nc.vector.tensor_scalar(
                                    se[:], io_s[:], de_s[:, col:col + 1],
                                    nv_s[:, col:col + 1], op0=eq, op1=mul)
                            col += 1
                            nc.tensor.matmul(psum_at[:], xr[:, off:off + F],
                                             se[:], start=(k == 0),
                                             stop=(k == tot - 1))
                        at_sb = atp.tile([128, 128], BF16)
                        nc.scalar.activation(at_sb[:], psum_at[:], ACopy)
                        nc.tensor.matmul(psum_h[:], at_sb[:],
                                         w1_s[:, r * H:(r + 1) * H],
                                         start=False, stop=False)
                    nc.tensor.matmul(psum_h[:], ones1[:], b1_s[:],
                                     start=False, stop=True)
                    h1_sb = h1p.tile([128, H], BF16)
                    nc.scalar.activation(h1_sb[:], psum_h[:], ARelu)
                    nc.sync.dma_start(h1loc[db * 128:(db + 1) * 128, :], h1_sb[:])
                    if _STAGE >= 2 and (db + 1) in ag_marks:
                        allgather_to(db + 1)
                    if _STAGE < 3:
                        sc_sb0 = h1p.tile([1, 128], F32, tag="scdbg")
                        nc.vector.tensor_copy(sc_sb0[:], h1_sb[0:1, 0:128])
                        nc.sync.dma_start(sc_d[0:1, db * 128:(db + 1) * 128],
                                          sc_sb0[:])

            # =============== ALLGATHER h1 (any remainder) ===============
            if _STAGE >= 2:
                allgather_to(NDB)

            # =============== LAYER 2 ===============
            if _STAGE >= 3:
              with (
                  tc.tile_pool(name="ring2", bufs=2) as ringp2,
                  tc.tile_pool(name="s2", bufs=8) as sp2,
                  tc.tile_pool(name="at2", bufs=4) as atp2,
                  tc.tile_pool(name="h2sb", bufs=2) as h2p,
                  tc.tile_pool(name="misc2", bufs=2) as mp2,
                  tc.tile_pool(name="pat2lo", bufs=2, space="PSUM") as patlo,
                  tc.tile_pool(name="pat2hi", bufs=2, space="PSUM") as pathi,
                  tc.tile_pool(name="ph2", bufs=2, space="PSUM") as php2,
                  tc.tile_pool(name="pmisc", bufs=1, space="PSUM") as pmp,
              ):
                  get2 = make_gather(ringp2, h1full[0:LO, :], h1full[HOFF:N, :],
                                     H, G2, 2)

                  col = 0
                  for db in range(NDB):
                      # root2 term needs h1^T of this dst block
                      h1row = mp2.tile([128, H], BF16, tag="h1row")
                      nc.sync.dma_start(h1row[:], h1loc[db * 128:(db + 1) * 128, :])
                      psum_h2 = php2.tile([128, O], F32)
                      h1t = []
                      for h in range(2):
                          pt = pmp.tile([128, 128], BF16, tag="ptr")
                          nc.tensor.transpose(pt[:], h1row[:, h * 128:(h + 1) * 128],
                                              id_s[:])
                          ht = mp2.tile([128, 128], BF16, tag=f"h1t{h}")
                          nc.scalar.activation(ht[:], pt[:], ACopy)
                          h1t.append(ht)
                      nc.tensor.matmul(psum_h2[:], h1t[0][:], r2_s[:, 0:O],
                                       start=True, stop=False)
                      nc.tensor.matmul(psum_h2[:], h1t[1][:], r2_s[:, O:2 * O],
                                       start=False, stop=False)

                      for r in range(R):
                          builds = _builds_for(sched, db, r)
                          tot = len(builds)
                          if tot == 0:
                              continue
                          at_lo = patlo.tile([128, 128], F32)
                          at_hi = pathi.tile([128, 128], F32)
                          for k, b in enumerate(builds):
                              xr, off = get2(b)
                              if _NOONEHOT:
                                  se = io_s
                              else:
                                  se = sp2.tile([128, 128], BF16, tag="se2")
                                  nc.vector.tensor_scalar(
                                      se[:], io_s[:], de_s[:, col:col + 1],
                                      nv_s[:, col:col + 1], op0=eq, op1=mul)
                              col += 1
                              st_f, sp_f = (k == 0), (k == tot - 1)
                              nc.tensor.matmul(at_lo[:], xr[:, off:off + 128],
                                               se[:], start=st_f, stop=sp_f)
                              nc.tensor.matmul(at_hi[:], xr[:, off + 128:off + 256],
                                               se[:], start=st_f, stop=sp_f)
                          at_sb = atp2.tile([128, 2 * 128], BF16)
                          nc.scalar.activation(at_sb[:, 0:128], at_lo[:], ACopy)
                          nc.scalar.activation(at_sb[:, 128:256], at_hi[:], ACopy)
                          for h in range(2):
                              nc.tensor.matmul(
                                  psum_h2[:], at_sb[:, h * 128:(h + 1) * 128],
                                  w2_s[:, (r * 2 + h) * O:(r * 2 + h + 1) * O],
                                  start=False, stop=False)
                      nc.tensor.matmul(psum_h2[:], ones1[:], b2_s[:],
                                       start=False, stop=True)
                      h2_sb = h2p.tile([128, O], BF16)
                      nc.scalar.activation(h2_sb[:], psum_h2[:], ARelu)

                      # head: scores = sigmoid(h2 @ Ws + bs)
                      pt2 = pmp.tile([128, 128], BF16, tag="ptr")
                      nc.tensor.transpose(pt2[:], h2_sb[:], id_s[:])
                      h2t = mp2.tile([128, 128], BF16, tag="h2t")
                      nc.scalar.activation(h2t[:], pt2[:], ACopy)
                      psc = pmp.tile([1, 128], F32, tag="psc")
                      nc.tensor.matmul(psc[:], ws_s[:], h2t[:], start=True, stop=True)
                      sc_sb = mp2.tile([1, 128], F32, tag="scsb")
                      nc.scalar.activation(sc_sb[:], psc[:], ASig,
                                           bias=bs_s[0:1, 0:1])
                      nc.sync.dma_start(sc_d[0:1, db * 128:(db + 1) * 128], sc_sb[:])

    nc.compile()
    return nc


# ---------------------------------------------------------------------------
# Entry point
# ---------------------------------------------------------------------------

def kernel(x, edge_index, edge_type, W1, root1, b1, W2, root2, b2, Ws, bs):
    x = np.ascontiguousarray(np.asarray(x, np.float32))
    ei = np.asarray(edge_index)
    et = np.asarray(edge_type).astype(np.int64)
    src, dst = ei[0].astype(np.int64), ei[1].astype(np.int64)
    W1 = np.asarray(W1, np.float32)
    root1 = np.ascontiguousarray(np.asarray(root1, np.float32))
    b1 = np.asarray(b1, np.float32)
    W2 = np.asarray(W2, np.float32)
    root2 = np.asarray(root2, np.float32)
    b2 = np.asarray(b2, np.float32)
    Ws = np.ascontiguousarray(np.asarray(Ws, np.float32))
    bs = np.asarray(bs, np.float32)

    N, F = x.shape
    R, _, H = W1.shape
    O = W2.shape[2]

    sched = _build_schedule(src, dst, et, N, R, NC)
    ND, NDB = sched["ND"], sched["NDB"]
    NDP = NDB * 128

    nc = _build_program(N, F, H, O, R, NC, sched)

    # common (replicated) inputs
    xbf = x.astype(NPBF)
    w1f = np.concatenate([W1[r] for r in range(R)], axis=1)            # [F, R*H]
    w2f = np.concatenate(
        [W2[r][h * 128:(h + 1) * 128, :] for r in range(R) for h in range(2)],
        axis=1)                                                         # [128, R*2*O]
    r2f = np.concatenate([root2[0:128, :], root2[128:256, :]], axis=1)  # [128, 2*O]
    iota = np.tile(np.arange(128, dtype=np.float32), (128, 1))
    ident = np.eye(128, dtype=np.float32)

    common = dict(
        x=np.ascontiguousarray(xbf),
        w1=np.ascontiguousarray(w1f.astype(NPBF)),
        w2=np.ascontiguousarray(w2f.astype(NPBF)),
        r1=root1.astype(NPBF), r2=np.ascontiguousarray(r2f.astype(NPBF)),
        b1=np.ascontiguousarray(b1.reshape(1, H).astype(NPBF)),
        b2=np.ascontiguousarray(b2.reshape(1, O).astype(NPBF)),
        ws=Ws.astype(NPBF), bs=np.ascontiguousarray(bs.reshape(1, 1)),
        iota=np.ascontiguousarray(iota.astype(NPBF)),
        ident=ident.astype(NPBF),
    )

    in_maps = []
    for c in range(NC):
        xt = np.zeros((128, NDP), NPBF)
        xt[:, :ND] = xbf[c * ND:(c + 1) * ND].T
        m = dict(common)
        m.update(
            xt=xt,
            idx16=np.ascontiguousarray(sched["cores"][c]["idx16"]),
            de=np.ascontiguousarray(sched["cores"][c]["de"]),
            nv=np.ascontiguousarray(sched["cores"][c]["nv"]),
        )
        in_maps.append(m)

    trace = bool(int(os.environ.get("K_TRACE", "0")))
    res = bass_utils.run_bass_kernel_spmd(nc, in_maps, core_ids=list(range(NC)),
                                          trace=trace)
    global last_exec_time_ns, last_results, last_nc, last_in_maps
    last_results = res
    last_exec_time_ns = res.exec_time_ns
    last_nc = nc
    last_in_maps = in_maps
    out = np.concatenate(
        [res.results[c]["scores"][0, :ND] for c in range(NC)])
    return out.astype(np.float32)


if __name__ == "__main__":
    import reference
    inputs = {k: np.asarray(v) for k, v in reference.setup_inputs().items()}
    got = kernel(**inputs)
    exp = np.asarray(reference.reference(**{k: v for k, v in reference.setup_inputs().items()}))
    err = np.abs(got - exp).max()
    rel = np.linalg.norm(got - exp) / np.linalg.norm(exp)
    print(f"max abs err {err:.3e}  rel {rel:.3e}")

